# revision 1
# baseline (speedup 1.0000x reference)
"""Trainium2 Bass kernel for nn_MultiHeadAttention_30374008717799.

Reference computation (per problem): q = k = v = x @ Wq.T reshaped to 16 heads
of dim 64; causal softmax attention with scale 1/sqrt(1024); output re-merged
to [B, S, 1024].

Sharding: 8 cores = 4 batches x 2 head-groups (8 heads each). Each core gets
x[b] ([2048, 1024]) and its 512 rows of Wq, and produces out[b, :, 512g:512g+512].
No collectives needed; host reassembles.

Per-core algorithm (matmuls in float32r -- full-rate fp32 streaming with
~1e-4 accuracy -- fp32 PSUM accumulation and fp32 softmax arithmetic):
 - Transpose x and Wq on-chip via PE (contraction must sit on partitions).
 - qT[d, s] = WqT.T @ xT   (d-major q, feeds both score operands)
 - q_SD[s, d] (+ appended ones column) via PE transposes of qT, feeds AV lhsT.
 - Since k == q, the unnormalized exp'd score matrix U = exp(s/32) is
   symmetric, so tiles of U^T (what the AV matmul needs as its moving operand)
   are computed directly as scores tiles in [k, q] orientation -- no
   per-tile transposes of probabilities.
 - Softmax denominators come for free: the AV stationary operand is
   [q_SD | ones] ([128, 65]), so PSUM row 64 accumulates Z_q.
 - ctxT tiles [65, 512] are PE-transposed back to [s, d] orientation and
   scaled by 1/Z (per-partition scalar) into the output tile.
"""

import numpy as np

import concourse.bass as bass
import concourse.mybir as mybir
import concourse.tile as tile
from concourse.tile import ScopedClock
from concourse.bass_utils import run_bass_kernel_spmd

F32 = mybir.dt.float32
BF16 = mybir.dt.bfloat16
F32R = mybir.dt.float32r
MM = F32R  # matmul dtype: f32r streams at bf16 rate (N>=256) with ~1.8e-4 accuracy
AF = mybir.ActivationFunctionType

S = 2048          # sequence length
E = 1024          # embed dim
DG = 512          # per-core output dims (8 heads x 64)
D = 64            # head dim
P = 128           # partitions
SC = S // P       # 16 s-chunks
EC = E // P       # 8 e-chunks
DC = DG // P      # 4 d-chunks (head pairs)
QB = S // 512     # 4 q-blocks of 512
SCALE = 1.0 / np.sqrt(1024.0)


class TC(tile.TileContext):
    """TileContext adapted to this walrus build, which caps sync-waits at ONE
    per instruction: extra waits are peeled onto same-engine NoOps emitted
    just before the overloaded instruction, and the final drain gets the same
    treatment."""

    MAX_WAITS = 1

    def _lower_ordered_insts(self, ordered):
        for bb_name, insts in ordered.items():
            new_list = []
            for inst in insts:
                si = inst.sync_info
                if si is not None and si.on_wait and len(si.on_wait) > 1:
                    waits = list(si.on_wait)
                    upds = list(si.on_update) if si.on_update else []
                    inst.sync_info = mybir.SyncInfo(
                        on_wait=waits[-1:], on_update=upds
                    )
                    for w in waits[:-1]:
                        nop = mybir.InstNoOp(
                            name=f"I-wsplit-{self.nc.next_id()}", ins=[], outs=[]
                        )
                        nop.engine = inst.engine
                        nop.sync_info = mybir.SyncInfo(on_wait=[w], on_update=[])
                        new_list.append(nop)
                new_list.append(inst)
            insts[:] = new_list
        return super()._lower_ordered_insts(ordered)

    def _drain_and_barrier(self, tick_clock, wait_clock):
        nc = self.nc
        drain_inst = nc.sync.drain()
        wait_clock.add_sem_waits(
            drain_inst.ins, ScopedClock({None: tick_clock.global_clock})
        )
        si = drain_inst.ins.sync_info
        waits = list(si.on_wait) if si is not None and si.on_wait else []
        upds = list(si.on_update) if si is not None and si.on_update else []
        if len(waits) > self.MAX_WAITS:
            drain_inst.ins.sync_info = mybir.SyncInfo(
                on_wait=waits[: self.MAX_WAITS], on_update=upds
            )
            rest = waits[self.MAX_WAITS:]
            for k in range(0, len(rest), self.MAX_WAITS):
                extra = nc.sync.drain()
                extra.ins.sync_info = mybir.SyncInfo(
                    on_wait=rest[k : k + self.MAX_WAITS], on_update=[]
                )
        nc.all_engine_barrier()
        popped = nc._tile_sem_poison_stack.pop()
        assert popped is self._sem_poison
        nc.clear_and_free_semaphores(list(self.sems.allocated().values()))
        nc.all_engine_barrier()


def build(reps=1):
    nc = bass.Bass("TRN2", target_bir_lowering=False, debug=False)
    x_d = nc.declare_dram_parameter("x", [S, E], F32, isOutput=False)
    wq_d = nc.declare_dram_parameter("wq", [DG, E], F32, isOutput=False)
    tri_d = nc.declare_dram_parameter("tri", [P, P], F32, isOutput=False)
    iden_d = nc.declare_dram_parameter("iden", [P, P], F32, isOutput=False)
    out_d = nc.declare_dram_parameter("out", [S, DG], F32, isOutput=True)

    from contextlib import ExitStack

    with TC(nc) as tc, ExitStack() as es:
        cpool = es.enter_context(tc.tile_pool(name="consts", bufs=1))
        big = es.enter_context(tc.tile_pool(name="big", bufs=1))
        ut_pool = es.enter_context(tc.tile_pool(name="ut", bufs=8))
        ep_pool = es.enter_context(tc.tile_pool(name="ep", bufs=4))
        rc_pool = es.enter_context(tc.tile_pool(name="rc", bufs=6))
        wpool = es.enter_context(tc.tile_pool(name="wt", bufs=1))
        xs_pool = es.enter_context(tc.tile_pool(name="xs", bufs=4))
        xt_pool = es.enter_context(tc.tile_pool(name="xt", bufs=2))
        psA = es.enter_context(tc.tile_pool(name="psA", bufs=2, space="PSUM"))
        psS = es.enter_context(tc.tile_pool(name="psS", bufs=2, space="PSUM"))
        psC = es.enter_context(tc.tile_pool(name="psC", bufs=2, space="PSUM"))

        # constants
        tri = cpool.tile([P, P], MM, name="tri")
        idf = cpool.tile([P, P], F32, name="idf")
        nc.sync.dma_start(idf[:], iden_d[:])
        trf = cpool.tile([P, P], F32, name="trf")
        nc.sync.dma_start(trf[:], tri_d[:])
        nc.vector.tensor_copy(tri[:], trf[:])  # round to MM dtype
        idm = cpool.tile([P, P], MM, name="idm")
        nc.vector.tensor_copy(idm[:], idf[:])
        zer = cpool.tile([P, P], F32, name="zer")
        nc.vector.memset(zer[:], 0.0)
        one16 = cpool.tile([P, SC], F32, name="one16")
        nc.vector.memset(one16[:], 1.0)
        o16 = one16[:].rearrange("p (a b) -> p a b", b=1)

        import contextlib
        loop_cm = tc.For_i(0, reps, 1) if reps > 1 else contextlib.nullcontext()
        es.enter_context(loop_cm)

        qT = big.tile([P, DC * S], MM, name="qT")
        # q_SD with ones column: layout [P, DC, SC, 130]:
        #   per (pair dc, k-chunk j): cols 0:64 head0 qsd, 64 ones,
        #                             65:129 head1 qsd, 129 ones
        qsd = big.tile([P, DC, SC, 130], MM, name="qsd")
        ctx_out = big.tile([P, SC * DG], F32, name="ctx_out")

        # ---- wq: load + transpose (once) ----
        wqT = wpool.tile([P, EC * DG], MM, name="wqT")
        for dc in range(DC):
            ws = xs_pool.tile([P, E], F32, name="ws", tag="xs")
            nc.sync.dma_start(ws[:], wq_d[dc * P : (dc + 1) * P, :])
            for ec in range(EC):
                pt = psA.tile([P, P], F32, name="ptw", tag="pt")
                nc.tensor.transpose(pt[:], ws[:, ec * P : (ec + 1) * P], idf[:])
                nc.vector.tensor_copy(
                    wqT[:, ec * DG + dc * P : ec * DG + (dc + 1) * P], pt[:]
                )

        # ---- fused pipeline over 512-wide s-blocks:
        #      transpose x block -> project q block -> attention for i=sb ----
        for sb in range(4):
            xtb = xt_pool.tile([P, EC * 512], MM, name="xtb")
            for sc4 in range(4):
                sc = 4 * sb + sc4
                xs = xs_pool.tile([P, E], F32, name="xs", tag="xs")
                nc.sync.dma_start(xs[:], x_d[sc * P : (sc + 1) * P, :])
                for ec in range(EC):
                    pt = psA.tile([P, P], F32, name="ptx", tag="pt")
                    nc.tensor.transpose(pt[:], xs[:, ec * P : (ec + 1) * P], idf[:])
                    nc.vector.tensor_copy(
                        xtb[:, ec * 512 + sc4 * P : ec * 512 + (sc4 + 1) * P], pt[:]
                    )
            for dc in range(DC):
                pq = psA.tile([P, 512], F32, name="pq", tag="pt")
                for ec in range(EC):
                    nc.tensor.matmul(
                        pq[:],
                        lhsT=wqT[:, ec * DG + dc * P : ec * DG + (dc + 1) * P],
                        rhs=xtb[:, ec * 512 : (ec + 1) * 512],
                        start=(ec == 0),
                        stop=(ec == EC - 1),
                    )
                nc.vector.tensor_copy(qT[:, dc * S + sb * 512 : dc * S + (sb + 1) * 512], pq[:])
                for j4 in range(4):
                    j = 4 * sb + j4
                    pt = psA.tile([P, P], MM, name="ptq", tag="pt")
                    nc.tensor.transpose(
                        pt[:], qT[:, dc * S + j * P : dc * S + (j + 1) * P], idm[:]
                    )
                    dst = qsd[:, dc, j].rearrange("p (g c) -> p g c", g=2)[:, :, 0:64]
                    srcp = pt[:].rearrange("p (g c) -> p g c", g=2)
                    nc.vector.tensor_copy(dst, srcp)
                nc.vector.tensor_copy(qsd[:, dc, 4 * sb : 4 * sb + 4, 64:65], o16[:, 0:4])
                nc.vector.tensor_copy(qsd[:, dc, 4 * sb : 4 * sb + 4, 129:130], o16[:, 0:4])

            # ---- attention for q-block i = sb (all 8 heads) ----
            # The two heads of a pair are interleaved inside the j-loop: their
            # K=64 score matmuls sit in disjoint PE row-groups (base partition
            # 0 vs 64) and issue back-to-back, so they run concurrently; the
            # per-head exp/AV chains ping-pong PE against ACT.
            i = sb
            njj = 4 * i + 4
            for dc in range(DC):
                cps = [psC.tile([P, 512], F32, name=f"cps{h2}", tag="cps") for h2 in range(2)]
                for j0 in range(0, njj, 2):
                    sts = []
                    for h2 in range(2):
                        pb = h2 * 64
                        st = psS.tile([P, 1024], F32, name="st", tag="st")
                        sts.append(st)
                        for u in range(2):
                            jj = j0 + u
                            off = u * 512
                            ce = min(max(0, jj * P - i * 512), 256)
                            nc.tensor.matmul(
                                st[:, off + ce : off + 512],
                                lhsT=qT[pb : pb + 64, dc * S + jj * P : dc * S + (jj + 1) * P],
                                rhs=qT[pb : pb + 64, dc * S + i * 512 + ce : dc * S + (i + 1) * 512],
                                start=True,
                                stop=True,
                            )
                    uts = []
                    ces = [min(max(0, (j0 + u) * P - i * 512), 256) for u in range(2)]
                    for h2 in range(2):
                        ut = ut_pool.tile([P, 1024], MM, name="ut")
                        uts.append(ut)
                        if ces[0] == 0 and ces[1] == 0:
                            nc.scalar.activation(ut[:], sts[h2][:], AF.Exp, scale=SCALE)
                        else:
                            for u in range(2):
                                o, ce = u * 512, ces[u]
                                nc.scalar.activation(
                                    ut[:, o + ce : o + 512],
                                    sts[h2][:, o + ce : o + 512],
                                    AF.Exp,
                                    scale=SCALE,
                                )
                    for h2 in range(2):
                        ut = uts[h2]
                        for u in range(2):
                            jj = j0 + u
                            off = u * 512
                            c0 = max(0, jj * P - i * 512)
                            ce = min(c0, 256)
                            if c0 > ce:
                                nc.vector.tensor_copy(
                                    ut[:, off + ce : off + c0], zer[:, 0 : c0 - ce]
                                )
                            if jj >= 4 * i:  # diagonal block: triangle mask
                                nc.vector.tensor_mul(
                                    ut[:, off + c0 : off + c0 + P],
                                    ut[:, off + c0 : off + c0 + P],
                                    tri[:],
                                )
                    for h2 in range(2):
                        for u in range(2):
                            jj = j0 + u
                            off = u * 512
                            ce = min(max(0, jj * P - i * 512), 256)
                            nc.tensor.matmul(
                                cps[h2][0:65, ce:512],
                                lhsT=qsd[:, dc, jj, h2 * 65 : h2 * 65 + 65],
                                rhs=uts[h2][:, off + ce : off + 512],
                                start=(jj == 0),
                                stop=(jj == njj - 1),
                            )
                # epilogue: transpose ctxT back to [s, d], normalize
                for h2 in range(2):
                    csb = ep_pool.tile([65, 512], F32, name="csb")
                    nc.vector.tensor_copy(csb[:], cps[h2][0:65, :])
                    for c in range(4):
                        sc = 4 * i + c
                        ptc = psA.tile([P, P], F32, name="ptc", tag="pt")
                        nc.tensor.transpose(
                            ptc[:, 0:65], csb[:, c * P : (c + 1) * P], idf[0:65, 0:65]
                        )
                        rc = rc_pool.tile([P, 1], F32, name="rc")
                        nc.vector.reciprocal(rc[:], ptc[:, 64:65])
                        h = 2 * dc + h2
                        nc.vector.tensor_scalar_mul(
                            ctx_out[:, sc * DG + h * D : sc * DG + (h + 1) * D],
                            ptc[:, 0:64],
                            rc[:],
                        )
            # ---- store this s-block\'s four 128-row chunks ----
            for c in range(4):
                sc = 4 * sb + c
                nc.sync.dma_start(
                    out_d[sc * P : (sc + 1) * P, :], ctx_out[:, sc * DG : (sc + 1) * DG]
                )

    return nc


def _host_consts():
    tri = np.triu(np.ones((P, P), dtype=np.float32))  # tri[k, q] = 1 iff k <= q
    iden = np.eye(P, dtype=np.float32)
    return tri, iden


def make_in_maps(x, Wq):
    tri, iden = _host_consts()
    in_maps = []
    for c in range(8):
        b, g = c // 2, c % 2
        in_maps.append(
            {
                "x": np.ascontiguousarray(np.asarray(x[b], dtype=np.float32)),
                "wq": np.ascontiguousarray(
                    np.asarray(Wq[g * DG : (g + 1) * DG], dtype=np.float32)
                ),
                "tri": tri,
                "iden": iden,
            }
        )
    return in_maps


_NC_CACHE = {}


def _get_nc():
    if "nc" not in _NC_CACHE:
        _NC_CACHE["nc"] = build()
    return _NC_CACHE["nc"]


def run(x, Wq, **spmd_kwargs):
    x = np.asarray(x, dtype=np.float32)
    Wq = np.asarray(Wq, dtype=np.float32)
    nc = _get_nc()
    in_maps = make_in_maps(x, Wq)
    kr = run_bass_kernel_spmd(nc, in_maps, list(range(8)), **spmd_kwargs)
    out = np.empty((4, S, E), dtype=np.float32)
    for c in range(8):
        b, g = c // 2, c % 2
        out[b, :, g * DG : (g + 1) * DG] = kr.results[c]["out"]
    return out.reshape(4, S, E), kr


def kernel(x, Wq):
    out, _ = run(x, Wq)
    return out

